# revision 7
# baseline (speedup 1.0000x reference)
"""Trainium2 Bass kernel for nn_Encoder_51900384804901.

6-layer post-norm TransformerEncoder (E=1024, NH=16, DFF=4096, relu FFN)
where every token attends only to the first num_ctx=1024 context tokens.

Sharding: data-parallel over batch. B=8 -> one batch element per NeuronCore,
no collectives. Each core runs the full 6-layer encoder on its [2048, 1024]
slice.

v2 design (vs the f32r baseline): everything the PE touches is fp16 (rel-err
budget 2e-2, measured ~5e-4 in host sim), the residual stream itself is fp16
feature-major x^T [E, T] resident in SBUF, and attention for q-chunk qc is
interleaved with the FFN of q-chunk qc-1 so the tensor engine never idles
long enough for the HAM clock gate to re-throttle (the f32r baseline spent
>60% of its matmuls at 1.2-1.95 GHz because the per-head softmax chain
stalled the PE every ~20us).

Per layer, per 512-token q-chunk, in issue order:
  Q-proj -> per head-pair: row-tiled quadrant scores (two K=64 matmuls run
  concurrently in the PE array), exp on ScalarE (scale=1/8 folded in, no max
  subtraction -- scores are bounded), PV in fp16 with a ones column per head
  producing the softmax denominator, fast-reciprocal (single custom DVE op)
  + partition_broadcast + one multiply -> out-proj accumulated into the
  residual -> LN1 -> FFN1(relu) -> FFN2 -> LN2.
K/V projection for layer l+1 issues after the last q-chunk's attention of
layer l, filling the layer boundary with dense GEMMs.

LayerNorm reductions run over the partition axis via ones-matmuls; rstd is
computed as exp(-0.5*ln(var+eps)) so the whole kernel uses one ScalarE
table set (natural_log_exp_and_others) -- zero table switches.

Self-contained: hardcodes all shapes; host pre-transposes weights to fp16.
"""

import os
import numpy as np

import concourse.bacc as bacc
import concourse.tile as tile
from concourse import mybir
from concourse import bass_utils

# Problem dims (hardcoded per contract)
L, E, NH, DFF = 6, 1024, 16, 4096
B, S, NC = 8, 2048, 1024
DH = E // NH  # 64
LN_EPS = 1e-5

F32 = mybir.dt.float32
FP16 = mybir.dt.float16

P = 128          # partitions
NQ = S // 512    # 4 q-chunks of 512
ET = E // P      # 8 e-tiles
FT = DFF // P    # 32 f-tiles
KT = NC // P     # 8 ctx k-tiles


def build_encoder():
    nc = bacc.Bacc("TRN2", debug=False)

    xT = nc.dram_tensor("xT", [E, S], FP16, kind="ExternalInput").ap()
    wqkvT = nc.dram_tensor("wqkvT", [L, E, 3 * E], FP16, kind="ExternalInput").ap()
    woT = nc.dram_tensor("woT", [L, E, E], FP16, kind="ExternalInput").ap()
    w1T = nc.dram_tensor("w1T", [L, E, DFF], FP16, kind="ExternalInput").ap()
    w2T = nc.dram_tensor("w2T", [L, DFF, E], FP16, kind="ExternalInput").ap()
    # params[l]: [128, 104] per-partition param columns:
    # 0-7 bq | 8-15 bk | 16-23 bv | 24-31 bo | 32-63 b1 | 64-71 b2
    # 72-79 g1 | 80-87 be1 | 88-95 g2 | 96-103 be2   (col m <-> e-tile m)
    params = nc.dram_tensor("params", [L, P, 104], F32, kind="ExternalInput").ap()
    bvrow = nc.dram_tensor("bvrow", [L, E], F32, kind="ExternalInput").ap()
    onesh = nc.dram_tensor("onesh", [P, 16], FP16, kind="ExternalInput").ap()
    outT = nc.dram_tensor("outT", [E, S], FP16, kind="ExternalOutput").ap()

    AF = mybir.ActivationFunctionType
    OP = mybir.AluOpType

    nlayers = int(os.environ.get("ENC_NLAYERS", L))
    skip_ffn = os.environ.get("ENC_SKIP_FFN", "") == "1"
    skip_attn = os.environ.get("ENC_SKIP_ATTN", "") == "1"
    skip_ln = os.environ.get("ENC_SKIP_LN", "") == "1"

    with tile.TileContext(nc) as tc:
        with tc.tile_pool(name="persist", bufs=1) as pp, \
             tc.tile_pool(name="kv", bufs=1) as kvp, \
             tc.tile_pool(name="qa", bufs=2) as qap, \
             tc.tile_pool(name="ao", bufs=1) as aop, \
             tc.tile_pool(name="hh", bufs=1) as hp, \
             tc.tile_pool(name="es", bufs=8) as esp, \
             tc.tile_pool(name="wst", bufs=6) as wst, \
             tc.tile_pool(name="kvw", bufs=1) as kvw, \
             tc.tile_pool(name="sc", bufs=2) as scp, \
             tc.tile_pool(name="lnp", bufs=2) as lnp, \
             tc.tile_pool(name="psc", bufs=2, space="PSUM") as psc, \
             tc.tile_pool(name="pj", bufs=3, space="PSUM") as pjp, \
             tc.tile_pool(name="ppo", bufs=1, space="PSUM") as ppo:

            # Residual stream x^T, resident fp16
            xt = [pp.tile([P, S], FP16, tag=f"x{i}", name=f"x{i}")
                  for i in range(ET)]
            for i in range(ET):
                nc.sync.dma_start(xt[i][:], xT[P * i:P * (i + 1), :])
            ones = pp.tile([P, 1], FP16, name="ones")
            nc.sync.dma_start(ones[:], onesh[:, 0:1])
            eps_t = pp.tile([P, 1], F32, tag="eps", name="eps")
            nc.vector.memset(eps_t[:], LN_EPS)

            # K^T feature-major + V' token-major (per layer, bufs=1: WAR deps
            # serialize next layer's KV proj behind this layer's attention)
            kt = [kvp.tile([P, NC], FP16, tag=f"k{i}", name=f"k{i}")
                  for i in range(ET)]
            vp = [kvp.tile([P, NH * (DH + 1)], FP16, tag=f"v{i}", name=f"v{i}")
                  for i in range(KT)]
            # ones columns of V' written once; V-proj evictions never touch them
            for t in range(KT):
                ones_cols = vp[t].rearrange("p (h c) -> p h c",
                                            c=DH + 1)[:, :, 64:65]
                nc.sync.dma_start(ones_cols, onesh[:, :, None])

            par_pool = [None]

            def gemm(wdram_l, col0, nk, rhs_tiles, mg_count, evict, wtag):
                """out[:, col0+mg*512 : ...] = sum_k W[k-tile].T @ rhs[k].
                Streams weights as [P,256] fp16 tiles (2 mi per DMA); psum
                pairs from the 3-buf "pj" pool. evict(mi_global, ps)."""
                for mg in range(mg_count):
                    for half in range(2):
                        c0 = col0 + mg * 512 + half * 256
                        ps = [pjp.tile([P, 512], F32, tag="pj", name="pj")
                              for _ in range(2)]
                        for k in range(nk):
                            w = wst.tile([P, 256], FP16, tag=wtag, name=wtag)
                            nc.sync.dma_start(
                                w[:], wdram_l[P * k:P * (k + 1), c0:c0 + 256])
                            for m2 in range(2):
                                nc.tensor.matmul(
                                    ps[m2][:], w[:, P * m2:P * (m2 + 1)],
                                    rhs_tiles[k],
                                    start=(k == 0), stop=(k == nk - 1))
                        for m2 in range(2):
                            evict(mg * 4 + half * 2 + m2, ps[m2])

            def kv_proj(l):
                """K^T (feature-major) and V' (token-major) for layer l."""
                par = par_pool[0]
                bvb = kvw.tile([P, E], F32, tag="bvb", name="bvb")
                bvr = kvw.tile([1, E], F32, tag="bvr", name="bvr")
                nc.sync.dma_start(bvr[:], bvrow[l][None, :])
                nc.gpsimd.partition_broadcast(bvb[:], bvr[:])
                for cc in range(2):
                    cs = slice(cc * 512, (cc + 1) * 512)
                    rhs = [xt[k][:, cs] for k in range(ET)]

                    def ev_k(m, ps, cs=cs):
                        nc.vector.tensor_scalar_add(
                            kt[m][:, cs], ps[:], par[:, 8 + m:9 + m])
                    gemm(wqkvT[l], E, ET, rhs, 2, ev_k, "wkv")
                # V token-major: lhsT = x ctx token tiles, rhs = wv tiles
                for ch in range(2):
                    wv = [kvw.tile([P, 512], FP16, tag=f"wv{k}",
                                   name=f"wv{k}") for k in range(ET)]
                    for k in range(ET):
                        nc.sync.dma_start(
                            wv[k][:], wqkvT[l, P * k:P * (k + 1),
                                            2 * E + ch * 512:
                                            2 * E + (ch + 1) * 512])
                    for t in range(KT):
                        ps = pjp.tile([P, 512], F32, tag="pj", name="pj")
                        for k in range(ET):
                            nc.tensor.matmul(
                                ps[:], xt[k][:, P * t:P * (t + 1)], wv[k][:],
                                start=(k == 0), stop=(k == ET - 1))
                        for hh in range(8):
                            h = ch * 8 + hh
                            nc.vector.tensor_tensor(
                                vp[t][:, h * 65:h * 65 + 64],
                                ps[:, hh * 64:(hh + 1) * 64],
                                bvb[:, h * 64:(h + 1) * 64], OP.add)

            def attention(l, qc):
                par = par_pool[0]
                cs = slice(qc * 512, (qc + 1) * 512)
                qt = [qap.tile([P, 512], FP16, tag=f"q{i}", name=f"q{i}")
                      for i in range(ET)]
                at = [aop.tile([P, 512], FP16, tag=f"a{i}", name=f"a{i}")
                      for i in range(ET)]
                rhs_x = [xt[k][:, cs] for k in range(ET)]

                def ev_q(m, ps):
                    nc.vector.tensor_scalar_add(qt[m][:], ps[:],
                                                par[:, m:m + 1])
                gemm(wqkvT[l], 0, ET, rhs_x, 2, ev_q, "w")

                for hp_ in range(NH // 2):
                    es8 = []
                    for t in range(KT):
                        pss = psc.tile([P, 1024], F32, tag="sc", name="sc")
                        for hi, off in ((0, 0), (1, 64)):
                            nc.tensor.matmul(
                                pss[:, hi * 512:(hi + 1) * 512],
                                kt[hp_][off:off + 64, P * t:P * (t + 1)],
                                qt[hp_][off:off + 64, :],
                                start=True, stop=True)
                        es = esp.tile([P, 1024], FP16, tag="es", name="es")
                        nc.scalar.activation(es[:], pss[:], AF.Exp,
                                             scale=0.125)
                        es8.append(es)
                    for hi in range(2):
                        h = 2 * hp_ + hi
                        po = ppo.tile([P, 512], F32, tag="po", name="po")
                        for t in range(KT):
                            nc.tensor.matmul(
                                po[0:DH + 1, :],
                                vp[t][:, h * 65:(h + 1) * 65],
                                es8[t][:, hi * 512:(hi + 1) * 512],
                                start=(t == 0), stop=(t == KT - 1))
                        rc = scp.tile([1, 512], F32, tag="rc", name="rc")
                        if os.environ.get("ENC_SLOW_RECIP", "") == "1":
                            nc.vector.reciprocal(rc[:], po[DH:DH + 1, :])
                        else:
                            # reciprocal_approx_fast from PSUM returns garbage
                            # (BITWISE_NOT seed path); stage the den row in
                            # SBUF first.
                            den = scp.tile([1, 512], F32, tag="den",
                                           name="den")
                            nc.vector.tensor_copy(den[:], po[DH:DH + 1, :])
                            nc.vector.reciprocal_approx_fast(
                                out=rc[:], in_=den[:])
                        bct = scp.tile([DH, 512], F32, tag="bct", name="bct")
                        nc.gpsimd.partition_broadcast(bct[:], rc[:])
                        nc.vector.tensor_tensor(
                            at[hp_][hi * 64:hi * 64 + 64, :],
                            po[0:DH, :], bct[:], OP.mult)

                def ev_o(m, ps):
                    tmp = scp.tile([P, 512], FP16, tag="tmp", name="tmp")
                    nc.vector.tensor_scalar_add(tmp[:], ps[:],
                                                par[:, 24 + m:25 + m])
                    nc.vector.tensor_tensor(xt[m][:, cs], xt[m][:, cs],
                                            tmp[:], OP.add)
                gemm(woT[l], 0, ET, [a[:] for a in at], 2, ev_o, "w")

            def layer_norm(qc, g_col, b_col):
                """Post-norm LN on xt[:, qc-chunk] in place (fp16)."""
                par = par_pool[0]
                cs = slice(qc * 512, (qc + 1) * 512)
                s1 = pjp.tile([P, 512], F32, tag="pj", name="pj")
                for k in range(ET):
                    nc.tensor.matmul(s1[0:1, :], ones[:], xt[k][:, cs],
                                     start=(k == 0), stop=(k == ET - 1))
                s2 = pjp.tile([P, 512], F32, tag="pj", name="pj")
                for k in range(ET):
                    sq = lnp.tile([P, 512], FP16, tag="sq", name="sq")
                    nc.vector.tensor_tensor(sq[:], xt[k][:, cs], xt[k][:, cs],
                                            OP.mult)
                    nc.tensor.matmul(s2[0:1, :], ones[:], sq[:],
                                     start=(k == 0), stop=(k == ET - 1))
                m1 = lnp.tile([1, 512], FP16, tag="m1", name="m1")
                nc.vector.tensor_scalar_mul(m1[:], s1[0:1, :], 1.0 / E)
                m2 = lnp.tile([1, 512], F32, tag="m2", name="m2")
                nc.vector.tensor_scalar_mul(m2[:], s2[0:1, :], 1.0 / E)
                # var = m2 - m1^2
                msq = lnp.tile([1, 512], F32, tag="msq", name="msq")
                nc.vector.tensor_tensor(msq[:], m1[:], m1[:], OP.mult)
                nc.vector.tensor_tensor(m2[:], m2[:], msq[:], OP.subtract)
                lnv = lnp.tile([1, 512], F32, tag="lnv", name="lnv")
                nc.scalar.activation(lnv[:], m2[:], AF.Ln, bias=eps_t[0:1, :])
                mb = lnp.tile([P, 512], FP16, tag="mb", name="mb")
                nc.gpsimd.partition_broadcast(mb[:], m1[:])
                lnb = lnp.tile([P, 512], F32, tag="lnb", name="lnb")
                nc.gpsimd.partition_broadcast(lnb[:], lnv[:])
                vb = lnp.tile([P, 512], FP16, tag="vb", name="vb")
                nc.scalar.activation(vb[:], lnb[:], AF.Exp, scale=-0.5)
                for k in range(ET):
                    t1 = lnp.tile([P, 512], FP16, tag="t1", name="t1")
                    nc.vector.tensor_tensor(t1[:], xt[k][:, cs], mb[:],
                                            OP.subtract)
                    nc.vector.tensor_tensor(t1[:], t1[:], vb[:], OP.mult)
                    nc.vector.tensor_scalar(
                        xt[k][:, cs], t1[:],
                        par[:, g_col + k:g_col + k + 1],
                        par[:, b_col + k:b_col + k + 1],
                        OP.mult, OP.add)

            def ffn(l, qc):
                par = par_pool[0]
                cs = slice(qc * 512, (qc + 1) * 512)
                ht = [hp.tile([P, 512], FP16, tag=f"h{i}", name=f"h{i}")
                      for i in range(FT)]
                rhs_x = [xt[k][:, cs] for k in range(ET)]

                def ev_h(m, ps):
                    nc.scalar.activation(ht[m][:], ps[:], AF.Relu,
                                         bias=par[:, 32 + m:33 + m])
                gemm(w1T[l], 0, ET, rhs_x, 8, ev_h, "w1")

                def ev_f2(m, ps):
                    tmp = scp.tile([P, 512], FP16, tag="tmp", name="tmp")
                    nc.vector.tensor_scalar_add(tmp[:], ps[:],
                                                par[:, 64 + m:65 + m])
                    nc.vector.tensor_tensor(xt[m][:, cs], xt[m][:, cs],
                                            tmp[:], OP.add)
                gemm(w2T[l], 0, FT, [h[:] for h in ht], 2, ev_f2, "w2")

            # ---------------- main schedule ----------------
            with tc.tile_pool(name="parp", bufs=2) as parp:
                def load_par(l):
                    par = parp.tile([P, 104], F32, tag="par", name="par")
                    nc.sync.dma_start(par[:], params[l])
                    par_pool[0] = par

                load_par(0)
                kv_proj(0)
                for l in range(nlayers):
                    for qc in range(NQ):
                        if not skip_attn:
                            attention(l, qc)
                        par_next = None
                        if qc == NQ - 1 and l + 1 < nlayers:
                            # K/V for next layer: needs ln2 of qc 0/1 (done),
                            # issues after last reader of this layer's kt/vp
                            par_l = par_pool[0]
                            load_par(l + 1)
                            par_next = par_pool[0]
                            kv_proj(l + 1)
                            par_pool[0] = par_l
                        if not skip_ln:
                            layer_norm(qc, 72, 80)
                        if not skip_ffn:
                            ffn(l, qc)
                            if not skip_ln:
                                layer_norm(qc, 88, 96)
                        if par_next is not None:
                            par_pool[0] = par_next

            for i in range(ET):
                nc.sync.dma_start(outT[P * i:P * (i + 1), :], xt[i][:])

    nc.compile()
    return nc


def _prep_inputs(inputs):
    """Host-side: transpose weights to fp16 / pack params; per-core in_maps."""
    emb = np.asarray(inputs["embeddings"], dtype=np.float32)
    ipw = np.asarray(inputs["in_proj_w"], dtype=np.float32)   # [L, 3E, E]
    ipb = np.asarray(inputs["in_proj_b"], dtype=np.float32)   # [L, 3E]
    ow = np.asarray(inputs["out_w"], dtype=np.float32)        # [L, E, E]
    ob = np.asarray(inputs["out_b"], dtype=np.float32)        # [L, E]
    l1w = np.asarray(inputs["lin1_w"], dtype=np.float32)      # [L, DFF, E]
    l1b = np.asarray(inputs["lin1_b"], dtype=np.float32)      # [L, DFF]
    l2w = np.asarray(inputs["lin2_w"], dtype=np.float32)      # [L, E, DFF]
    l2b = np.asarray(inputs["lin2_b"], dtype=np.float32)      # [L, E]
    g1 = np.asarray(inputs["ln1_w"], dtype=np.float32)
    be1 = np.asarray(inputs["ln1_b"], dtype=np.float32)
    g2 = np.asarray(inputs["ln2_w"], dtype=np.float32)
    be2 = np.asarray(inputs["ln2_b"], dtype=np.float32)

    wqkvT = np.ascontiguousarray(ipw.transpose(0, 2, 1)).astype(np.float16)
    woT = np.ascontiguousarray(ow.transpose(0, 2, 1)).astype(np.float16)
    w1T = np.ascontiguousarray(l1w.transpose(0, 2, 1)).astype(np.float16)
    w2T = np.ascontiguousarray(l2w.transpose(0, 2, 1)).astype(np.float16)

    def cols(a, n):  # [L, n*128] -> [L, 128, n]
        return a.reshape(L, n, P).transpose(0, 2, 1)

    params = np.concatenate([
        cols(ipb[:, 0:E], 8), cols(ipb[:, E:2 * E], 8), cols(ipb[:, 2 * E:], 8),
        cols(ob, 8), cols(l1b, 32), cols(l2b, 8),
        cols(g1, 8), cols(be1, 8), cols(g2, 8), cols(be2, 8),
    ], axis=2)
    params = np.ascontiguousarray(params, dtype=np.float32)   # [L, 128, 104]
    bvrow = np.ascontiguousarray(ipb[:, 2 * E:3 * E])         # [L, E]

    shared = dict(wqkvT=wqkvT, woT=woT, w1T=w1T, w2T=w2T,
                  params=params, bvrow=bvrow,
                  onesh=np.ones((P, 16), np.float16))
    in_maps = []
    for c in range(B):
        m = dict(shared)
        m["xT"] = np.ascontiguousarray(emb[c].T).astype(np.float16)  # [E, S]
        in_maps.append(m)
    return in_maps


_NC_CACHE = {}


def _get_nc():
    if "nc" not in _NC_CACHE:
        _NC_CACHE["nc"] = build_encoder()
    return _NC_CACHE["nc"]


def run(inputs, trace=False, tmpdir=None):
    """Run on 8 NeuronCores; returns (output [8, S, E], BassKernelResults)."""
    in_maps = _prep_inputs(inputs)
    nc = _get_nc()
    res = bass_utils.run_bass_kernel_spmd(
        nc, in_maps, core_ids=list(range(B)), trace=trace, tmpdir=tmpdir)
    out = np.stack([np.ascontiguousarray(res.results[c]["outT"].T)
                    for c in range(B)]).astype(np.float32)
    return out, res


def kernel(**inputs):
    num_ctx = int(np.asarray(inputs["num_ctx"]))
    assert num_ctx == NC, f"kernel hardcodes num_ctx={NC}, got {num_ctx}"
    out, _ = run(inputs)
    return out


# revision 11
# speedup vs baseline: 1.2146x; 1.2146x over previous
"""Trainium2 Bass kernel for nn_Encoder_51900384804901.

6-layer post-norm TransformerEncoder (E=1024, NH=16, DFF=4096, relu FFN)
where every token attends only to the first num_ctx=1024 context tokens.

Sharding: data-parallel over batch. B=8 -> one batch element per NeuronCore,
no collectives. Each core runs the full 6-layer encoder on its [2048, 1024]
slice.

v2 design (vs the f32r baseline): everything the PE touches is fp16 (rel-err
budget 2e-2, measured ~5e-4 in host sim), the residual stream itself is fp16
feature-major x^T [E, T] resident in SBUF, and attention for q-chunk qc is
interleaved with the FFN of q-chunk qc-1 so the tensor engine never idles
long enough for the HAM clock gate to re-throttle (the f32r baseline spent
>60% of its matmuls at 1.2-1.95 GHz because the per-head softmax chain
stalled the PE every ~20us).

Per layer, per 512-token q-chunk, in issue order:
  Q-proj -> per head-pair: row-tiled quadrant scores (two K=64 matmuls run
  concurrently in the PE array), exp on ScalarE (scale=1/8 folded in, no max
  subtraction -- scores are bounded), PV in fp16 with a ones column per head
  producing the softmax denominator, fast-reciprocal (single custom DVE op)
  + partition_broadcast + one multiply -> out-proj accumulated into the
  residual -> LN1 -> FFN1(relu) -> FFN2 -> LN2.
K/V projection for layer l+1 issues after the last q-chunk's attention of
layer l, filling the layer boundary with dense GEMMs.

LayerNorm reductions run over the partition axis via ones-matmuls; rstd is
computed as exp(-0.5*ln(var+eps)) so the whole kernel uses one ScalarE
table set (natural_log_exp_and_others) -- zero table switches.

Self-contained: hardcodes all shapes; host pre-transposes weights to fp16.
"""

import os
import numpy as np

import concourse.bacc as bacc
import concourse.tile as tile
from concourse import mybir
from concourse import bass_utils

# Problem dims (hardcoded per contract)
L, E, NH, DFF = 6, 1024, 16, 4096
B, S, NC = 8, 2048, 1024
DH = E // NH  # 64
LN_EPS = 1e-5

F32 = mybir.dt.float32
FP16 = mybir.dt.float16

P = 128          # partitions
NQ = S // 512    # 4 q-chunks of 512
ET = E // P      # 8 e-tiles
FT = DFF // P    # 32 f-tiles
KT = NC // P     # 8 ctx k-tiles


def build_encoder():
    nc = bacc.Bacc("TRN2", debug=False)

    xT = nc.dram_tensor("xT", [E, S], FP16, kind="ExternalInput").ap()
    wqkvT = nc.dram_tensor("wqkvT", [L, E, 3 * E], FP16, kind="ExternalInput").ap()
    woT = nc.dram_tensor("woT", [L, E, E], FP16, kind="ExternalInput").ap()
    w1T = nc.dram_tensor("w1T", [L, E, DFF], FP16, kind="ExternalInput").ap()
    w2T = nc.dram_tensor("w2T", [L, DFF, E], FP16, kind="ExternalInput").ap()
    # params[l]: [128, 104] per-partition param columns:
    # 0-7 bq | 8-15 bk | 16-23 bv | 24-31 bo | 32-63 b1 | 64-71 b2
    # 72-79 g1 | 80-87 be1 | 88-95 g2 | 96-103 be2   (col m <-> e-tile m)
    params = nc.dram_tensor("params", [L, P, 104], F32, kind="ExternalInput").ap()
    bvrow = nc.dram_tensor("bvrow", [L, E], F32, kind="ExternalInput").ap()
    onesh = nc.dram_tensor("onesh", [P, 16], FP16, kind="ExternalInput").ap()
    outT = nc.dram_tensor("outT", [E, S], FP16, kind="ExternalOutput").ap()

    AF = mybir.ActivationFunctionType
    OP = mybir.AluOpType

    nlayers = int(os.environ.get("ENC_NLAYERS", L))
    skip_ffn = os.environ.get("ENC_SKIP_FFN", "") == "1"
    skip_attn = os.environ.get("ENC_SKIP_ATTN", "") == "1"
    skip_ln = os.environ.get("ENC_SKIP_LN", "") == "1"

    with tile.TileContext(nc) as tc:
        with tc.tile_pool(name="persist", bufs=1) as pp, \
             tc.tile_pool(name="kv", bufs=1) as kvp, \
             tc.tile_pool(name="qa", bufs=2) as qap, \
             tc.tile_pool(name="ao", bufs=1) as aop, \
             tc.tile_pool(name="hh", bufs=1) as hp, \
             tc.tile_pool(name="es", bufs=8) as esp, \
             tc.tile_pool(name="wst", bufs=9) as wst, \
             tc.tile_pool(name="kvw", bufs=1) as kvw, \
             tc.tile_pool(name="sc", bufs=2) as scp, \
             tc.tile_pool(name="lnp", bufs=2) as lnp, \
             tc.tile_pool(name="psc", bufs=2, space="PSUM") as psc, \
             tc.tile_pool(name="pj", bufs=3, space="PSUM") as pjp, \
             tc.tile_pool(name="ppo", bufs=1, space="PSUM") as ppo:

            # Residual stream x^T, resident fp16
            xt = [pp.tile([P, S], FP16, tag=f"x{i}", name=f"x{i}")
                  for i in range(ET)]
            for i in range(ET):
                nc.sync.dma_start(xt[i][:], xT[P * i:P * (i + 1), :])
            ones = pp.tile([P, 1], FP16, name="ones")
            nc.sync.dma_start(ones[:], onesh[:, 0:1])
            eps_t = pp.tile([P, 1], F32, tag="eps", name="eps")
            nc.vector.memset(eps_t[:], LN_EPS)

            # K^T feature-major + V' token-major (per layer, bufs=1: WAR deps
            # serialize next layer's KV proj behind this layer's attention)
            kt = [kvp.tile([P, NC], FP16, tag=f"k{i}", name=f"k{i}")
                  for i in range(ET)]
            vp = [kvp.tile([P, NH * (DH + 1)], FP16, tag=f"v{i}", name=f"v{i}")
                  for i in range(KT)]
            # ones columns of V' written once; V-proj evictions never touch them
            for t in range(KT):
                ones_cols = vp[t].rearrange("p (h c) -> p h c",
                                            c=DH + 1)[:, :, 64:65]
                nc.sync.dma_start(ones_cols, onesh[:, :, None])

            par_pool = [None]

            def gemm(wdram_l, col0, nk, rhs_tiles, mg_count, evict, wtag):
                """out[:, col0+mg*512 : ...] = sum_k W[k-tile].T @ rhs[k].
                psum comes in mi-pairs from the 3-buf "pj" pool;
                evict(mi_global, ps). nk<=8: one [P,512] weight DMA feeds 4
                matmuls (both mi-pair passes); nk>8 (FFN2): [P,256] stream."""
                for mg in range(mg_count):
                    if nk <= 8:
                        wt = [wst.tile([P, 512], FP16, tag=wtag, name=wtag)
                              for _ in range(nk)]
                        for k in range(nk):
                            nc.sync.dma_start(
                                wt[k][:],
                                wdram_l[P * k:P * (k + 1),
                                        col0 + mg * 512:col0 + (mg + 1) * 512])
                        for half in range(2):
                            ps = [pjp.tile([P, 512], F32, tag="pj", name="pj")
                                  for _ in range(2)]
                            for k in range(nk):
                                for m2 in range(2):
                                    mi = half * 256 + P * m2
                                    nc.tensor.matmul(
                                        ps[m2][:], wt[k][:, mi:mi + P],
                                        rhs_tiles[k],
                                        start=(k == 0), stop=(k == nk - 1))
                            for m2 in range(2):
                                evict(mg * 4 + half * 2 + m2, ps[m2])
                    else:
                        for half in range(2):
                            c0 = col0 + mg * 512 + half * 256
                            ps = [pjp.tile([P, 512], F32, tag="pj", name="pj")
                                  for _ in range(2)]
                            for k in range(nk):
                                w = wst.tile([P, 256], FP16, tag=wtag,
                                             name=wtag)
                                nc.sync.dma_start(
                                    w[:],
                                    wdram_l[P * k:P * (k + 1), c0:c0 + 256])
                                for m2 in range(2):
                                    nc.tensor.matmul(
                                        ps[m2][:], w[:, P * m2:P * (m2 + 1)],
                                        rhs_tiles[k],
                                        start=(k == 0), stop=(k == nk - 1))
                            for m2 in range(2):
                                evict(mg * 4 + half * 2 + m2, ps[m2])

            def kv_proj(l):
                """K^T (feature-major) and V' (token-major) for layer l."""
                par = par_pool[0]
                for cc in range(2):
                    cs = slice(cc * 512, (cc + 1) * 512)
                    rhs = [xt[k][:, cs] for k in range(ET)]

                    def ev_k(m, ps, cs=cs):
                        nc.vector.tensor_scalar_add(
                            kt[m][:, cs], ps[:], par[:, 8 + m:9 + m])
                    gemm(wqkvT[l], E, ET, rhs, 2, ev_k, "wa")
                # V token-major: lhsT = x ctx token tiles, rhs = wv tiles
                for ch in range(2):
                    bvb = kvw.tile([P, 512], F32, tag="bvb", name="bvb")
                    bvr = kvw.tile([1, 512], F32, tag="bvr", name="bvr")
                    nc.sync.dma_start(
                        bvr[:], bvrow[l][None, ch * 512:(ch + 1) * 512])
                    nc.gpsimd.partition_broadcast(bvb[:], bvr[:])
                    wv = [kvw.tile([P, 512], FP16, tag=f"wv{k}",
                                   name=f"wv{k}") for k in range(ET)]
                    for k in range(ET):
                        nc.sync.dma_start(
                            wv[k][:], wqkvT[l, P * k:P * (k + 1),
                                            2 * E + ch * 512:
                                            2 * E + (ch + 1) * 512])
                    for t in range(KT):
                        ps = pjp.tile([P, 512], F32, tag="pj", name="pj")
                        for k in range(ET):
                            nc.tensor.matmul(
                                ps[:], xt[k][:, P * t:P * (t + 1)], wv[k][:],
                                start=(k == 0), stop=(k == ET - 1))
                        for hh in range(8):
                            h = ch * 8 + hh
                            nc.vector.tensor_tensor(
                                vp[t][:, h * 65:h * 65 + 64],
                                ps[:, hh * 64:(hh + 1) * 64],
                                bvb[:, hh * 64:(hh + 1) * 64], OP.add)

            def attention(l, qc):
                par = par_pool[0]
                cs = slice(qc * 512, (qc + 1) * 512)
                qt = [qap.tile([P, 512], FP16, tag=f"q{i}", name=f"q{i}")
                      for i in range(ET)]
                at = [aop.tile([P, 512], FP16, tag=f"a{i}", name=f"a{i}")
                      for i in range(ET)]
                rhs_x = [xt[k][:, cs] for k in range(ET)]

                def ev_q(m, ps):
                    nc.vector.tensor_scalar_add(qt[m][:], ps[:],
                                                par[:, m:m + 1])
                gemm(wqkvT[l], 0, ET, rhs_x, 2, ev_q, "wa")

                for hp_ in range(NH // 2):
                    es8 = []
                    for t in range(KT):
                        pss = psc.tile([P, 1024], F32, tag="sc", name="sc")
                        for hi, off in ((0, 0), (1, 64)):
                            nc.tensor.matmul(
                                pss[:, hi * 512:(hi + 1) * 512],
                                kt[hp_][off:off + 64, P * t:P * (t + 1)],
                                qt[hp_][off:off + 64, :],
                                start=True, stop=True)
                        es = esp.tile([P, 1024], FP16, tag="es", name="es")
                        nc.scalar.activation(es[:], pss[:], AF.Exp,
                                             scale=0.125)
                        es8.append(es)
                    for hi in range(2):
                        h = 2 * hp_ + hi
                        po = ppo.tile([P, 512], F32, tag="po", name="po")
                        for t in range(KT):
                            nc.tensor.matmul(
                                po[0:DH + 1, :],
                                vp[t][:, h * 65:(h + 1) * 65],
                                es8[t][:, hi * 512:(hi + 1) * 512],
                                start=(t == 0), stop=(t == KT - 1))
                        # two copies drain the bank; recip needs an SBUF
                        # input at base_partition 0 (else garbage)
                        poS = scp.tile([DH, 512], F32, tag="poS", name="poS")
                        nc.vector.tensor_copy(poS[:], po[0:DH, :])
                        den = scp.tile([1, 512], F32, tag="den", name="den")
                        nc.vector.tensor_copy(den[:], po[DH:DH + 1, :])
                        rc = scp.tile([1, 512], F32, tag="rc", name="rc")
                        nc.vector.reciprocal_approx_fast(
                            out=rc[:], in_=den[:])
                        bct = scp.tile([DH, 512], F32, tag="bct", name="bct")
                        nc.gpsimd.partition_broadcast(bct[:], rc[:])
                        nc.vector.tensor_tensor(
                            at[hp_][hi * 64:hi * 64 + 64, :],
                            poS[:], bct[:], OP.mult)

                def ev_o(m, ps):
                    tmp = scp.tile([P, 512], FP16, tag="tmp", name="tmp")
                    nc.vector.tensor_scalar_add(tmp[:], ps[:],
                                                par[:, 24 + m:25 + m])
                    nc.vector.tensor_tensor(xt[m][:, cs], xt[m][:, cs],
                                            tmp[:], OP.add)
                gemm(woT[l], 0, ET, [a[:] for a in at], 2, ev_o, "wa")

            def layer_norm(qc, g_col, b_col):
                """Post-norm LN on xt[:, qc-chunk] in place (fp16)."""
                par = par_pool[0]
                cs = slice(qc * 512, (qc + 1) * 512)
                s1 = pjp.tile([P, 512], F32, tag="pj", name="pj")
                for k in range(ET):
                    nc.tensor.matmul(s1[0:1, :], ones[:], xt[k][:, cs],
                                     start=(k == 0), stop=(k == ET - 1))
                s2 = pjp.tile([P, 512], F32, tag="pj", name="pj")
                for k in range(ET):
                    sq = lnp.tile([P, 512], FP16, tag="sq", name="sq")
                    nc.scalar.activation(sq[:], xt[k][:, cs], AF.Square)
                    nc.tensor.matmul(s2[0:1, :], ones[:], sq[:],
                                     start=(k == 0), stop=(k == ET - 1))
                m1 = lnp.tile([1, 512], FP16, tag="m1", name="m1", bufs=1)
                nc.vector.tensor_scalar_mul(m1[:], s1[0:1, :], 1.0 / E)
                m2 = lnp.tile([1, 512], F32, tag="m2", name="m2", bufs=1)
                nc.vector.tensor_scalar_mul(m2[:], s2[0:1, :], 1.0 / E)
                # var = m2 - m1^2
                msq = lnp.tile([1, 512], F32, tag="msq", name="msq", bufs=1)
                nc.vector.tensor_tensor(msq[:], m1[:], m1[:], OP.mult)
                nc.vector.tensor_tensor(m2[:], m2[:], msq[:], OP.subtract)
                lnv = lnp.tile([1, 512], F32, tag="lnv", name="lnv", bufs=1)
                nc.scalar.activation(lnv[:], m2[:], AF.Ln, bias=eps_t[0:1, :])
                mb = lnp.tile([P, 512], FP16, tag="mb", name="mb")
                nc.gpsimd.partition_broadcast(mb[:], m1[:])
                lnb = lnp.tile([P, 512], F32, tag="lnb", name="lnb")
                nc.gpsimd.partition_broadcast(lnb[:], lnv[:])
                vb = lnp.tile([P, 512], FP16, tag="vb", name="vb")
                nc.scalar.activation(vb[:], lnb[:], AF.Exp, scale=-0.5)
                for k in range(ET):
                    t1 = lnp.tile([P, 512], FP16, tag="t1", name="t1")
                    nc.vector.tensor_tensor(t1[:], xt[k][:, cs], mb[:],
                                            OP.subtract)
                    nc.vector.tensor_tensor(t1[:], t1[:], vb[:], OP.mult)
                    nc.vector.tensor_scalar(
                        xt[k][:, cs], t1[:],
                        par[:, g_col + k:g_col + k + 1],
                        par[:, b_col + k:b_col + k + 1],
                        OP.mult, OP.add)

            def ffn(l, qc):
                par = par_pool[0]
                cs = slice(qc * 512, (qc + 1) * 512)
                ht = [hp.tile([P, 512], FP16, tag=f"h{i}", name=f"h{i}")
                      for i in range(FT)]
                rhs_x = [xt[k][:, cs] for k in range(ET)]

                def ev_h(m, ps):
                    nc.scalar.activation(ht[m][:], ps[:], AF.Relu,
                                         bias=par[:, 32 + m:33 + m])
                gemm(w1T[l], 0, ET, rhs_x, 8, ev_h, "wf")

                def ev_f2(m, ps):
                    tmp = scp.tile([P, 512], FP16, tag="tmp", name="tmp")
                    nc.vector.tensor_scalar_add(tmp[:], ps[:],
                                                par[:, 64 + m:65 + m])
                    nc.vector.tensor_tensor(xt[m][:, cs], xt[m][:, cs],
                                            tmp[:], OP.add)
                gemm(w2T[l], 0, FT, [h[:] for h in ht], 2, ev_f2, "wf")

            # ---------------- main schedule ----------------
            with tc.tile_pool(name="parp", bufs=2) as parp:
                def load_par(l):
                    par = parp.tile([P, 104], F32, tag="par", name="par")
                    nc.sync.dma_start(par[:], params[l])
                    par_pool[0] = par

                load_par(0)
                kv_proj(0)
                for l in range(nlayers):
                    for qc in range(NQ):
                        if not skip_attn:
                            attention(l, qc)
                        par_next = None
                        if qc == NQ - 1 and l + 1 < nlayers:
                            # K/V for next layer: needs ln2 of qc 0/1 (done),
                            # issues after last reader of this layer's kt/vp
                            par_l = par_pool[0]
                            load_par(l + 1)
                            par_next = par_pool[0]
                            kv_proj(l + 1)
                            par_pool[0] = par_l
                        if not skip_ln:
                            layer_norm(qc, 72, 80)
                        if not skip_ffn:
                            ffn(l, qc)
                            if not skip_ln:
                                layer_norm(qc, 88, 96)
                        if par_next is not None:
                            par_pool[0] = par_next

            for i in range(ET):
                nc.sync.dma_start(outT[P * i:P * (i + 1), :], xt[i][:])

    nc.compile()
    return nc


def _prep_inputs(inputs):
    """Host-side: transpose weights to fp16 / pack params; per-core in_maps."""
    emb = np.asarray(inputs["embeddings"], dtype=np.float32)
    ipw = np.asarray(inputs["in_proj_w"], dtype=np.float32)   # [L, 3E, E]
    ipb = np.asarray(inputs["in_proj_b"], dtype=np.float32)   # [L, 3E]
    ow = np.asarray(inputs["out_w"], dtype=np.float32)        # [L, E, E]
    ob = np.asarray(inputs["out_b"], dtype=np.float32)        # [L, E]
    l1w = np.asarray(inputs["lin1_w"], dtype=np.float32)      # [L, DFF, E]
    l1b = np.asarray(inputs["lin1_b"], dtype=np.float32)      # [L, DFF]
    l2w = np.asarray(inputs["lin2_w"], dtype=np.float32)      # [L, E, DFF]
    l2b = np.asarray(inputs["lin2_b"], dtype=np.float32)      # [L, E]
    g1 = np.asarray(inputs["ln1_w"], dtype=np.float32)
    be1 = np.asarray(inputs["ln1_b"], dtype=np.float32)
    g2 = np.asarray(inputs["ln2_w"], dtype=np.float32)
    be2 = np.asarray(inputs["ln2_b"], dtype=np.float32)

    wqkvT = np.ascontiguousarray(ipw.transpose(0, 2, 1)).astype(np.float16)
    woT = np.ascontiguousarray(ow.transpose(0, 2, 1)).astype(np.float16)
    w1T = np.ascontiguousarray(l1w.transpose(0, 2, 1)).astype(np.float16)
    w2T = np.ascontiguousarray(l2w.transpose(0, 2, 1)).astype(np.float16)

    def cols(a, n):  # [L, n*128] -> [L, 128, n]
        return a.reshape(L, n, P).transpose(0, 2, 1)

    params = np.concatenate([
        cols(ipb[:, 0:E], 8), cols(ipb[:, E:2 * E], 8), cols(ipb[:, 2 * E:], 8),
        cols(ob, 8), cols(l1b, 32), cols(l2b, 8),
        cols(g1, 8), cols(be1, 8), cols(g2, 8), cols(be2, 8),
    ], axis=2)
    params = np.ascontiguousarray(params, dtype=np.float32)   # [L, 128, 104]
    bvrow = np.ascontiguousarray(ipb[:, 2 * E:3 * E])         # [L, E]

    shared = dict(wqkvT=wqkvT, woT=woT, w1T=w1T, w2T=w2T,
                  params=params, bvrow=bvrow,
                  onesh=np.ones((P, 16), np.float16))
    in_maps = []
    for c in range(B):
        m = dict(shared)
        m["xT"] = np.ascontiguousarray(emb[c].T).astype(np.float16)  # [E, S]
        in_maps.append(m)
    return in_maps


_NC_CACHE = {}


def _get_nc():
    if "nc" not in _NC_CACHE:
        _NC_CACHE["nc"] = build_encoder()
    return _NC_CACHE["nc"]


def run(inputs, trace=False, tmpdir=None):
    """Run on 8 NeuronCores; returns (output [8, S, E], BassKernelResults)."""
    in_maps = _prep_inputs(inputs)
    nc = _get_nc()
    res = bass_utils.run_bass_kernel_spmd(
        nc, in_maps, core_ids=list(range(B)), trace=trace, tmpdir=tmpdir)
    out = np.stack([np.ascontiguousarray(res.results[c]["outT"].T)
                    for c in range(B)]).astype(np.float32)
    return out, res


def kernel(**inputs):
    num_ctx = int(np.asarray(inputs["num_ctx"]))
    assert num_ctx == NC, f"kernel hardcodes num_ctx={NC}, got {num_ctx}"
    out, _ = run(inputs)
    return out


# revision 15
# speedup vs baseline: 1.3519x; 1.1131x over previous
"""Trainium2 Bass kernel for nn_Encoder_51900384804901.

6-layer post-norm TransformerEncoder (E=1024, NH=16, DFF=4096, relu FFN)
where every token attends only to the first num_ctx=1024 context tokens.

Sharding: data-parallel over batch. B=8 -> one batch element per NeuronCore,
no collectives. Each core runs the full 6-layer encoder on its [2048, 1024]
slice.

v2 design (vs the f32r baseline): everything the PE touches is fp16 (rel-err
budget 2e-2, measured ~5e-4 in host sim), the residual stream itself is fp16
feature-major x^T [E, T] resident in SBUF, and attention for q-chunk qc is
interleaved with the FFN of q-chunk qc-1 so the tensor engine never idles
long enough for the HAM clock gate to re-throttle (the f32r baseline spent
>60% of its matmuls at 1.2-1.95 GHz because the per-head softmax chain
stalled the PE every ~20us).

Per layer, per 512-token q-chunk, in issue order:
  Q-proj -> per head-pair: row-tiled quadrant scores (two K=64 matmuls run
  concurrently in the PE array), exp on ScalarE (scale=1/8 folded in, no max
  subtraction -- scores are bounded), PV in fp16 with a ones column per head
  producing the softmax denominator, fast-reciprocal (single custom DVE op)
  + partition_broadcast + one multiply -> out-proj accumulated into the
  residual -> LN1 -> FFN1(relu) -> FFN2 -> LN2.
K/V projection for layer l+1 issues after the last q-chunk's attention of
layer l, filling the layer boundary with dense GEMMs.

LayerNorm reductions run over the partition axis via ones-matmuls; rstd is
computed as exp(-0.5*ln(var+eps)) so the whole kernel uses one ScalarE
table set (natural_log_exp_and_others) -- zero table switches.

Self-contained: hardcodes all shapes; host pre-transposes weights to fp16.
"""

import os
import numpy as np

import concourse.bacc as bacc
import concourse.tile as tile
from concourse import mybir
from concourse import bass_utils

# Problem dims (hardcoded per contract)
L, E, NH, DFF = 6, 1024, 16, 4096
B, S, NC = 8, 2048, 1024
DH = E // NH  # 64
LN_EPS = 1e-5

F32 = mybir.dt.float32
FP16 = mybir.dt.float16

P = 128          # partitions
NQ = S // 512    # 4 q-chunks of 512
ET = E // P      # 8 e-tiles
FT = DFF // P    # 32 f-tiles
KT = NC // P     # 8 ctx k-tiles


def build_encoder():
    nc = bacc.Bacc("TRN2", debug=False)

    xT = nc.dram_tensor("xT", [E, S], FP16, kind="ExternalInput").ap()
    wqkvT = nc.dram_tensor("wqkvT", [L, E, 3 * E], FP16, kind="ExternalInput").ap()
    woT = nc.dram_tensor("woT", [L, E, E], FP16, kind="ExternalInput").ap()
    w1T = nc.dram_tensor("w1T", [L, E, DFF], FP16, kind="ExternalInput").ap()
    w2T = nc.dram_tensor("w2T", [L, DFF, E], FP16, kind="ExternalInput").ap()
    # params[l]: [128, 104] per-partition param columns:
    # 0-7 bq | 8-15 bk | 16-23 bv | 24-31 bo | 32-63 b1 | 64-71 b2
    # 72-79 g1 | 80-87 be1 | 88-95 g2 | 96-103 be2   (col m <-> e-tile m)
    params = nc.dram_tensor("params", [L, P, 104], F32, kind="ExternalInput").ap()
    bvrow = nc.dram_tensor("bvrow", [L, E], F32, kind="ExternalInput").ap()
    onesh = nc.dram_tensor("onesh", [P, 16], FP16, kind="ExternalInput").ap()
    outT = nc.dram_tensor("outT", [E, S], FP16, kind="ExternalOutput").ap()

    AF = mybir.ActivationFunctionType
    OP = mybir.AluOpType

    nlayers = int(os.environ.get("ENC_NLAYERS", L))
    skip_ffn = os.environ.get("ENC_SKIP_FFN", "") == "1"
    skip_attn = os.environ.get("ENC_SKIP_ATTN", "") == "1"
    skip_ln = os.environ.get("ENC_SKIP_LN", "") == "1"

    with tile.TileContext(nc) as tc:
        with tc.tile_pool(name="persist", bufs=1) as pp, \
             tc.tile_pool(name="kv", bufs=1) as kvp, \
             tc.tile_pool(name="qa", bufs=2) as qap, \
             tc.tile_pool(name="ao", bufs=1) as aop, \
             tc.tile_pool(name="hh", bufs=1) as hp, \
             tc.tile_pool(name="es", bufs=8) as esp, \
             tc.tile_pool(name="wst", bufs=2) as wst, \
             tc.tile_pool(name="kvw", bufs=1) as kvw, \
             tc.tile_pool(name="sc", bufs=2) as scp, \
             tc.tile_pool(name="lnp", bufs=2) as lnp, \
             tc.tile_pool(name="psc", bufs=2, space="PSUM") as psc, \
             tc.tile_pool(name="pj", bufs=3, space="PSUM") as pjp, \
             tc.tile_pool(name="ppo", bufs=1, space="PSUM") as ppo:

            # Residual stream x^T, resident fp16
            xt = [pp.tile([P, S], FP16, tag=f"x{i}", name=f"x{i}")
                  for i in range(ET)]
            for i in range(ET):
                nc.sync.dma_start(xt[i][:], xT[P * i:P * (i + 1), :])
            ones = pp.tile([P, 1], FP16, name="ones")
            nc.sync.dma_start(ones[:], onesh[:, 0:1])
            eps_t = pp.tile([P, 1], F32, tag="eps", name="eps")
            nc.vector.memset(eps_t[:], LN_EPS)

            # K^T feature-major + V' token-major (per layer, bufs=1: WAR deps
            # serialize next layer's KV proj behind this layer's attention)
            kt = [kvp.tile([P, NC], FP16, tag=f"k{i}", name=f"k{i}")
                  for i in range(ET)]
            vp = [kvp.tile([P, NH * (DH + 1)], FP16, tag=f"v{i}", name=f"v{i}")
                  for i in range(KT)]
            # ones columns of V' written once; V-proj evictions never touch them
            for t in range(KT):
                ones_cols = vp[t].rearrange("p (h c) -> p h c",
                                            c=DH + 1)[:, :, 64:65]
                nc.sync.dma_start(ones_cols, onesh[:, :, None])

            par_pool = [None]

            def gemm(wdram_l, col0, nk, rhs_tiles, mg_count, evict, wtag):
                """out[:, col0+mg*512 : ...] = sum_k W[k-tile].T @ rhs[k].
                psum comes in mi-pairs from the 3-buf "pj" pool;
                evict(mi_global, ps). Weights arrive in [P, <=8, 256] batched
                DMAs -- one descriptor feeds up to 16 matmuls."""
                wre = wdram_l.rearrange("(t p) c -> p t c", p=P)
                for mg in range(mg_count):
                    for half in range(2):
                        c0 = col0 + mg * 512 + half * 256
                        ps = [pjp.tile([P, 512], F32, tag="pj", name="pj")
                              for _ in range(2)]
                        for kc in range(0, nk, 8):
                            kn = min(8, nk - kc)
                            wt = wst.tile([P, 8, 256], FP16, tag=wtag,
                                          name=wtag)
                            nc.sync.dma_start(
                                wt[:, 0:kn, :], wre[:, kc:kc + kn, c0:c0 + 256])
                            for k in range(kn):
                                for m2 in range(2):
                                    nc.tensor.matmul(
                                        ps[m2][:], wt[:, k, P * m2:P * (m2 + 1)],
                                        rhs_tiles[kc + k],
                                        start=(kc + k == 0),
                                        stop=(kc + k == nk - 1))
                        for m2 in range(2):
                            evict(mg * 4 + half * 2 + m2, ps[m2])

            def kv_proj(l):
                """K^T (feature-major) and V' (token-major) for layer l."""
                par = par_pool[0]
                for cc in range(2):
                    cs = slice(cc * 512, (cc + 1) * 512)
                    rhs = [xt[k][:, cs] for k in range(ET)]

                    def ev_k(m, ps, cs=cs):
                        nc.vector.tensor_scalar_add(
                            kt[m][:, cs], ps[:], par[:, 8 + m:9 + m])
                    gemm(wqkvT[l], E, ET, rhs, 2, ev_k, "wa")
                # V token-major: lhsT = x ctx token tiles, rhs = wv tiles
                for ch in range(2):
                    bvb = kvw.tile([P, 512], F32, tag="bvb", name="bvb")
                    bvr = kvw.tile([1, 512], F32, tag="bvr", name="bvr")
                    nc.sync.dma_start(
                        bvr[:], bvrow[l][None, ch * 512:(ch + 1) * 512])
                    nc.gpsimd.partition_broadcast(bvb[:], bvr[:])
                    wv = kvw.tile([P, ET, 512], FP16, tag="wv", name="wv")
                    nc.sync.dma_start(
                        wv[:], wqkvT[l].rearrange("(t p) c -> p t c", p=P)
                        [:, :, 2 * E + ch * 512:2 * E + (ch + 1) * 512])
                    for t in range(KT):
                        ps = pjp.tile([P, 512], F32, tag="pj", name="pj")
                        for k in range(ET):
                            nc.tensor.matmul(
                                ps[:], xt[k][:, P * t:P * (t + 1)],
                                wv[:, k, :],
                                start=(k == 0), stop=(k == ET - 1))
                        for hh in range(8):
                            h = ch * 8 + hh
                            nc.vector.tensor_tensor(
                                vp[t][:, h * 65:h * 65 + 64],
                                ps[:, hh * 64:(hh + 1) * 64],
                                bvb[:, hh * 64:(hh + 1) * 64], OP.add)

            def attention(l, qc):
                par = par_pool[0]
                cs = slice(qc * 512, (qc + 1) * 512)
                qt = [qap.tile([P, 512], FP16, tag=f"q{i}", name=f"q{i}")
                      for i in range(ET)]
                at = [aop.tile([P, 512], FP16, tag=f"a{i}", name=f"a{i}")
                      for i in range(ET)]
                rhs_x = [xt[k][:, cs] for k in range(ET)]

                def ev_q(m, ps):
                    nc.vector.tensor_scalar_add(qt[m][:], ps[:],
                                                par[:, m:m + 1])
                gemm(wqkvT[l], 0, ET, rhs_x, 2, ev_q, "wa")

                for hp_ in range(NH // 2):
                    es8 = []
                    for t in range(KT):
                        pss = psc.tile([P, 1024], F32, tag="sc", name="sc")
                        for hi, off in ((0, 0), (1, 64)):
                            nc.tensor.matmul(
                                pss[:, hi * 512:(hi + 1) * 512],
                                kt[hp_][off:off + 64, P * t:P * (t + 1)],
                                qt[hp_][off:off + 64, :],
                                start=True, stop=True)
                        es = esp.tile([P, 1024], FP16, tag="es", name="es")
                        nc.scalar.activation(es[:], pss[:], AF.Exp,
                                             scale=0.125)
                        es8.append(es)
                    for hi in range(2):
                        h = 2 * hp_ + hi
                        po = ppo.tile([P, 512], F32, tag="po", name="po")
                        for t in range(KT):
                            nc.tensor.matmul(
                                po[0:DH + 1, :],
                                vp[t][:, h * 65:(h + 1) * 65],
                                es8[t][:, hi * 512:(hi + 1) * 512],
                                start=(t == 0), stop=(t == KT - 1))
                        # two copies drain the bank; recip needs an SBUF
                        # input at base_partition 0 (else garbage)
                        poS = scp.tile([DH, 512], F32, tag="poS", name="poS")
                        nc.vector.tensor_copy(poS[:], po[0:DH, :])
                        den = scp.tile([1, 512], F32, tag="den", name="den")
                        nc.vector.tensor_copy(den[:], po[DH:DH + 1, :])
                        rc = scp.tile([1, 512], F32, tag="rc", name="rc")
                        nc.vector.reciprocal_approx_fast(
                            out=rc[:], in_=den[:])
                        bct = scp.tile([DH, 512], F32, tag="bct", name="bct")
                        nc.gpsimd.partition_broadcast(bct[:], rc[:])
                        nc.vector.tensor_tensor(
                            at[hp_][hi * 64:hi * 64 + 64, :],
                            poS[:], bct[:], OP.mult)

                def ev_o(m, ps):
                    tmp = scp.tile([P, 512], FP16, tag="tmp", name="tmp")
                    nc.vector.tensor_scalar_add(tmp[:], ps[:],
                                                par[:, 24 + m:25 + m])
                    nc.vector.tensor_tensor(xt[m][:, cs], xt[m][:, cs],
                                            tmp[:], OP.add)
                gemm(woT[l], 0, ET, [a[:] for a in at], 2, ev_o, "wa")

            def layer_norm(qc, g_col, b_col):
                """Post-norm LN on xt[:, qc-chunk] in place (fp16)."""
                par = par_pool[0]
                cs = slice(qc * 512, (qc + 1) * 512)
                s1 = pjp.tile([P, 512], F32, tag="pj", name="pj")
                for k in range(ET):
                    nc.tensor.matmul(s1[0:1, :], ones[:], xt[k][:, cs],
                                     start=(k == 0), stop=(k == ET - 1))
                s2 = pjp.tile([P, 512], F32, tag="pj", name="pj")
                for k in range(ET):
                    sq = lnp.tile([P, 512], FP16, tag="sq", name="sq")
                    nc.scalar.activation(sq[:], xt[k][:, cs], AF.Square)
                    nc.tensor.matmul(s2[0:1, :], ones[:], sq[:],
                                     start=(k == 0), stop=(k == ET - 1))
                m1 = lnp.tile([1, 512], FP16, tag="m1", name="m1", bufs=1)
                nc.vector.tensor_scalar_mul(m1[:], s1[0:1, :], 1.0 / E)
                m2 = lnp.tile([1, 512], F32, tag="m2", name="m2", bufs=1)
                nc.vector.tensor_scalar_mul(m2[:], s2[0:1, :], 1.0 / E)
                # var = m2 - m1^2
                msq = lnp.tile([1, 512], F32, tag="msq", name="msq", bufs=1)
                nc.vector.tensor_tensor(msq[:], m1[:], m1[:], OP.mult)
                nc.vector.tensor_tensor(m2[:], m2[:], msq[:], OP.subtract)
                lnv = lnp.tile([1, 512], F32, tag="lnv", name="lnv", bufs=1)
                nc.scalar.activation(lnv[:], m2[:], AF.Ln, bias=eps_t[0:1, :])
                mb = lnp.tile([P, 512], FP16, tag="mb", name="mb")
                nc.gpsimd.partition_broadcast(mb[:], m1[:])
                lnb = lnp.tile([P, 512], F32, tag="lnb", name="lnb")
                nc.gpsimd.partition_broadcast(lnb[:], lnv[:])
                vb = lnp.tile([P, 512], FP16, tag="vb", name="vb")
                nc.scalar.activation(vb[:], lnb[:], AF.Exp, scale=-0.5)
                for k in range(ET):
                    t1 = lnp.tile([P, 512], FP16, tag="t1", name="t1")
                    nc.vector.tensor_tensor(t1[:], xt[k][:, cs], mb[:],
                                            OP.subtract)
                    nc.vector.tensor_tensor(t1[:], t1[:], vb[:], OP.mult)
                    nc.vector.tensor_scalar(
                        xt[k][:, cs], t1[:],
                        par[:, g_col + k:g_col + k + 1],
                        par[:, b_col + k:b_col + k + 1],
                        OP.mult, OP.add)

            def ffn(l, qc):
                par = par_pool[0]
                cs = slice(qc * 512, (qc + 1) * 512)
                ht = [hp.tile([P, 512], FP16, tag=f"h{i}", name=f"h{i}")
                      for i in range(FT)]
                rhs_x = [xt[k][:, cs] for k in range(ET)]

                def ev_h(m, ps):
                    nc.scalar.activation(ht[m][:], ps[:], AF.Relu,
                                         bias=par[:, 32 + m:33 + m])
                gemm(w1T[l], 0, ET, rhs_x, 8, ev_h, "wf")

                def ev_f2(m, ps):
                    tmp = scp.tile([P, 512], FP16, tag="tmp", name="tmp")
                    nc.vector.tensor_scalar_add(tmp[:], ps[:],
                                                par[:, 64 + m:65 + m])
                    nc.vector.tensor_tensor(xt[m][:, cs], xt[m][:, cs],
                                            tmp[:], OP.add)
                gemm(w2T[l], 0, FT, [h[:] for h in ht], 2, ev_f2, "wf")

            # ---------------- main schedule ----------------
            with tc.tile_pool(name="parp", bufs=2) as parp:
                def load_par(l):
                    par = parp.tile([P, 104], F32, tag="par", name="par")
                    nc.sync.dma_start(par[:], params[l])
                    par_pool[0] = par

                load_par(0)
                kv_proj(0)
                simple = skip_attn or skip_ffn or skip_ln
                if simple:
                    for l in range(nlayers):
                        for qc in range(NQ):
                            if not skip_attn:
                                attention(l, qc)
                            par_next = None
                            if qc == NQ - 1 and l + 1 < nlayers:
                                par_l = par_pool[0]
                                load_par(l + 1)
                                par_next = par_pool[0]
                                kv_proj(l + 1)
                                par_pool[0] = par_l
                            if not skip_ln:
                                layer_norm(qc, 72, 80)
                            if not skip_ffn:
                                ffn(l, qc)
                                if not skip_ln:
                                    layer_norm(qc, 88, 96)
                            if par_next is not None:
                                par_pool[0] = par_next
                else:
                    # software pipeline: attention(next) issues ahead of
                    # ffn(cur) so FFN matmuls fill attention's exp/PV stalls
                    attention(0, 0)
                    for l in range(nlayers):
                        for qc in range(NQ):
                            par_next = None
                            if qc + 1 < NQ:
                                attention(l, qc + 1)
                            elif l + 1 < nlayers:
                                par_l = par_pool[0]
                                load_par(l + 1)
                                par_next = par_pool[0]
                                kv_proj(l + 1)
                                attention(l + 1, 0)
                                par_pool[0] = par_l
                            layer_norm(qc, 72, 80)
                            ffn(l, qc)
                            layer_norm(qc, 88, 96)
                            if par_next is not None:
                                par_pool[0] = par_next

            for i in range(ET):
                nc.sync.dma_start(outT[P * i:P * (i + 1), :], xt[i][:])

    nc.compile()
    return nc


def _prep_inputs(inputs):
    """Host-side: transpose weights to fp16 / pack params; per-core in_maps."""
    emb = np.asarray(inputs["embeddings"], dtype=np.float32)
    ipw = np.asarray(inputs["in_proj_w"], dtype=np.float32)   # [L, 3E, E]
    ipb = np.asarray(inputs["in_proj_b"], dtype=np.float32)   # [L, 3E]
    ow = np.asarray(inputs["out_w"], dtype=np.float32)        # [L, E, E]
    ob = np.asarray(inputs["out_b"], dtype=np.float32)        # [L, E]
    l1w = np.asarray(inputs["lin1_w"], dtype=np.float32)      # [L, DFF, E]
    l1b = np.asarray(inputs["lin1_b"], dtype=np.float32)      # [L, DFF]
    l2w = np.asarray(inputs["lin2_w"], dtype=np.float32)      # [L, E, DFF]
    l2b = np.asarray(inputs["lin2_b"], dtype=np.float32)      # [L, E]
    g1 = np.asarray(inputs["ln1_w"], dtype=np.float32)
    be1 = np.asarray(inputs["ln1_b"], dtype=np.float32)
    g2 = np.asarray(inputs["ln2_w"], dtype=np.float32)
    be2 = np.asarray(inputs["ln2_b"], dtype=np.float32)

    wqkvT = np.ascontiguousarray(ipw.transpose(0, 2, 1)).astype(np.float16)
    woT = np.ascontiguousarray(ow.transpose(0, 2, 1)).astype(np.float16)
    w1T = np.ascontiguousarray(l1w.transpose(0, 2, 1)).astype(np.float16)
    w2T = np.ascontiguousarray(l2w.transpose(0, 2, 1)).astype(np.float16)

    def cols(a, n):  # [L, n*128] -> [L, 128, n]
        return a.reshape(L, n, P).transpose(0, 2, 1)

    params = np.concatenate([
        cols(ipb[:, 0:E], 8), cols(ipb[:, E:2 * E], 8), cols(ipb[:, 2 * E:], 8),
        cols(ob, 8), cols(l1b, 32), cols(l2b, 8),
        cols(g1, 8), cols(be1, 8), cols(g2, 8), cols(be2, 8),
    ], axis=2)
    params = np.ascontiguousarray(params, dtype=np.float32)   # [L, 128, 104]
    bvrow = np.ascontiguousarray(ipb[:, 2 * E:3 * E])         # [L, E]

    shared = dict(wqkvT=wqkvT, woT=woT, w1T=w1T, w2T=w2T,
                  params=params, bvrow=bvrow,
                  onesh=np.ones((P, 16), np.float16))
    in_maps = []
    for c in range(B):
        m = dict(shared)
        m["xT"] = np.ascontiguousarray(emb[c].T).astype(np.float16)  # [E, S]
        in_maps.append(m)
    return in_maps


_NC_CACHE = {}


def _get_nc():
    if "nc" not in _NC_CACHE:
        _NC_CACHE["nc"] = build_encoder()
    return _NC_CACHE["nc"]


def run(inputs, trace=False, tmpdir=None):
    """Run on 8 NeuronCores; returns (output [8, S, E], BassKernelResults)."""
    in_maps = _prep_inputs(inputs)
    nc = _get_nc()
    res = bass_utils.run_bass_kernel_spmd(
        nc, in_maps, core_ids=list(range(B)), trace=trace, tmpdir=tmpdir)
    out = np.stack([np.ascontiguousarray(res.results[c]["outT"].T)
                    for c in range(B)]).astype(np.float32)
    return out, res


def kernel(**inputs):
    num_ctx = int(np.asarray(inputs["num_ctx"]))
    assert num_ctx == NC, f"kernel hardcodes num_ctx={NC}, got {num_ctx}"
    out, _ = run(inputs)
    return out


# revision 16
# speedup vs baseline: 1.3643x; 1.0092x over previous
"""Trainium2 Bass kernel for nn_Encoder_51900384804901.

6-layer post-norm TransformerEncoder (E=1024, NH=16, DFF=4096, relu FFN)
where every token attends only to the first num_ctx=1024 context tokens.

Sharding: data-parallel over batch. B=8 -> one batch element per NeuronCore,
no collectives. Each core runs the full 6-layer encoder on its [2048, 1024]
slice.

v2 design (vs the f32r baseline): everything the PE touches is fp16 (rel-err
budget 2e-2, measured ~5e-4 in host sim), the residual stream itself is fp16
feature-major x^T [E, T] resident in SBUF, and attention for q-chunk qc is
interleaved with the FFN of q-chunk qc-1 so the tensor engine never idles
long enough for the HAM clock gate to re-throttle (the f32r baseline spent
>60% of its matmuls at 1.2-1.95 GHz because the per-head softmax chain
stalled the PE every ~20us).

Per layer, per 512-token q-chunk, in issue order:
  Q-proj -> per head-pair: row-tiled quadrant scores (two K=64 matmuls run
  concurrently in the PE array), exp on ScalarE (scale=1/8 folded in, no max
  subtraction -- scores are bounded), PV in fp16 with a ones column per head
  producing the softmax denominator, fast-reciprocal (single custom DVE op)
  + partition_broadcast + one multiply -> out-proj accumulated into the
  residual -> LN1 -> FFN1(relu) -> FFN2 -> LN2.
K/V projection for layer l+1 issues after the last q-chunk's attention of
layer l, filling the layer boundary with dense GEMMs.

LayerNorm reductions run over the partition axis via ones-matmuls; rstd is
computed as exp(-0.5*ln(var+eps)) so the whole kernel uses one ScalarE
table set (natural_log_exp_and_others) -- zero table switches.

Self-contained: hardcodes all shapes; host pre-transposes weights to fp16.
"""

import os
import numpy as np

import concourse.bacc as bacc
import concourse.tile as tile
from concourse import mybir
from concourse import bass_utils

# Problem dims (hardcoded per contract)
L, E, NH, DFF = 6, 1024, 16, 4096
B, S, NC = 8, 2048, 1024
DH = E // NH  # 64
LN_EPS = 1e-5

F32 = mybir.dt.float32
FP16 = mybir.dt.float16

P = 128          # partitions
NQ = S // 512    # 4 q-chunks of 512
ET = E // P      # 8 e-tiles
FT = DFF // P    # 32 f-tiles
KT = NC // P     # 8 ctx k-tiles


def build_encoder():
    nc = bacc.Bacc("TRN2", debug=False)

    xT = nc.dram_tensor("xT", [E, S], FP16, kind="ExternalInput").ap()
    wqkvT = nc.dram_tensor("wqkvT", [L, E, 3 * E], FP16, kind="ExternalInput").ap()
    woT = nc.dram_tensor("woT", [L, E, E], FP16, kind="ExternalInput").ap()
    w1T = nc.dram_tensor("w1T", [L, E, DFF], FP16, kind="ExternalInput").ap()
    w2T = nc.dram_tensor("w2T", [L, DFF, E], FP16, kind="ExternalInput").ap()
    # params[l]: [128, 104] per-partition param columns:
    # 0-7 bq | 8-15 bk | 16-23 bv | 24-31 bo | 32-63 b1 | 64-71 b2
    # 72-79 g1 | 80-87 be1 | 88-95 g2 | 96-103 be2   (col m <-> e-tile m)
    params = nc.dram_tensor("params", [L, P, 104], F32, kind="ExternalInput").ap()
    bvrow = nc.dram_tensor("bvrow", [L, E], F32, kind="ExternalInput").ap()
    onesh = nc.dram_tensor("onesh", [P, 16], FP16, kind="ExternalInput").ap()
    outT = nc.dram_tensor("outT", [E, S], FP16, kind="ExternalOutput").ap()

    AF = mybir.ActivationFunctionType
    OP = mybir.AluOpType

    nlayers = int(os.environ.get("ENC_NLAYERS", L))
    skip_ffn = os.environ.get("ENC_SKIP_FFN", "") == "1"
    skip_attn = os.environ.get("ENC_SKIP_ATTN", "") == "1"
    skip_ln = os.environ.get("ENC_SKIP_LN", "") == "1"

    with tile.TileContext(nc) as tc:
        with tc.tile_pool(name="persist", bufs=1) as pp, \
             tc.tile_pool(name="kv", bufs=1) as kvp, \
             tc.tile_pool(name="qa", bufs=2) as qap, \
             tc.tile_pool(name="ao", bufs=1) as aop, \
             tc.tile_pool(name="hh", bufs=1) as hp, \
             tc.tile_pool(name="es", bufs=8) as esp, \
             tc.tile_pool(name="wst", bufs=2) as wst, \
             tc.tile_pool(name="kvw", bufs=1) as kvw, \
             tc.tile_pool(name="sc", bufs=2) as scp, \
             tc.tile_pool(name="lnp", bufs=2) as lnp, \
             tc.tile_pool(name="psc", bufs=2, space="PSUM") as psc, \
             tc.tile_pool(name="pj", bufs=3, space="PSUM") as pjp, \
             tc.tile_pool(name="ppo", bufs=1, space="PSUM") as ppo:

            # Residual stream x^T, resident fp16
            xt = [pp.tile([P, S], FP16, tag=f"x{i}", name=f"x{i}")
                  for i in range(ET)]
            for i in range(ET):
                nc.sync.dma_start(xt[i][:], xT[P * i:P * (i + 1), :])
            ones = pp.tile([P, 1], FP16, name="ones")
            nc.sync.dma_start(ones[:], onesh[:, 0:1])
            eps_t = pp.tile([P, 1], F32, tag="eps", name="eps")
            nc.vector.memset(eps_t[:], LN_EPS)

            # K^T feature-major + V' token-major (per layer, bufs=1: WAR deps
            # serialize next layer's KV proj behind this layer's attention)
            kt = [kvp.tile([P, NC], FP16, tag=f"k{i}", name=f"k{i}")
                  for i in range(ET)]
            vp = [kvp.tile([P, NH * (DH + 1)], FP16, tag=f"v{i}", name=f"v{i}")
                  for i in range(KT)]
            # ones columns of V' written once; V-proj evictions never touch them
            for t in range(KT):
                ones_cols = vp[t].rearrange("p (h c) -> p h c",
                                            c=DH + 1)[:, :, 64:65]
                nc.sync.dma_start(ones_cols, onesh[:, :, None])

            par_pool = [None]

            def gemm(wdram_l, col0, nk, rhs_tiles, mg_count, evict, wtag):
                """out[:, col0+mg*512 : ...] = sum_k W[k-tile].T @ rhs[k].
                psum comes in mi-pairs from the 3-buf "pj" pool;
                evict(mi_global, ps). Weights arrive in [P, <=8, 256] batched
                DMAs -- one descriptor feeds up to 16 matmuls."""
                wre = wdram_l.rearrange("(t p) c -> p t c", p=P)
                for mg in range(mg_count):
                    for half in range(2):
                        c0 = col0 + mg * 512 + half * 256
                        ps = [pjp.tile([P, 512], F32, tag="pj", name="pj")
                              for _ in range(2)]
                        for kc in range(0, nk, 8):
                            kn = min(8, nk - kc)
                            wt = wst.tile([P, 8, 256], FP16, tag=wtag,
                                          name=wtag)
                            nc.sync.dma_start(
                                wt[:, 0:kn, :], wre[:, kc:kc + kn, c0:c0 + 256])
                            for k in range(kn):
                                for m2 in range(2):
                                    nc.tensor.matmul(
                                        ps[m2][:], wt[:, k, P * m2:P * (m2 + 1)],
                                        rhs_tiles[kc + k],
                                        start=(kc + k == 0),
                                        stop=(kc + k == nk - 1))
                        for m2 in range(2):
                            evict(mg * 4 + half * 2 + m2, ps[m2])

            def kv_proj(l):
                """K^T (feature-major) and V' (token-major) for layer l."""
                par = par_pool[0]
                for cc in range(2):
                    cs = slice(cc * 512, (cc + 1) * 512)
                    rhs = [xt[k][:, cs] for k in range(ET)]

                    def ev_k(m, ps, cs=cs):
                        nc.vector.tensor_scalar_add(
                            kt[m][:, cs], ps[:], par[:, 8 + m:9 + m])
                    gemm(wqkvT[l], E, ET, rhs, 2, ev_k, "wa")
                # V token-major: lhsT = x ctx token tiles, rhs = wv tiles
                for ch in range(2):
                    bvb = kvw.tile([P, 512], F32, tag="bvb", name="bvb")
                    bvr = kvw.tile([1, 512], F32, tag="bvr", name="bvr")
                    nc.sync.dma_start(
                        bvr[:], bvrow[l][None, ch * 512:(ch + 1) * 512])
                    nc.gpsimd.partition_broadcast(bvb[:], bvr[:])
                    wv = kvw.tile([P, ET, 512], FP16, tag="wv", name="wv")
                    nc.sync.dma_start(
                        wv[:], wqkvT[l].rearrange("(t p) c -> p t c", p=P)
                        [:, :, 2 * E + ch * 512:2 * E + (ch + 1) * 512])
                    for t in range(KT):
                        ps = pjp.tile([P, 512], F32, tag="pj", name="pj")
                        for k in range(ET):
                            nc.tensor.matmul(
                                ps[:], xt[k][:, P * t:P * (t + 1)],
                                wv[:, k, :],
                                start=(k == 0), stop=(k == ET - 1))
                        for hh in range(8):
                            h = ch * 8 + hh
                            nc.vector.tensor_tensor(
                                vp[t][:, h * 65:h * 65 + 64],
                                ps[:, hh * 64:(hh + 1) * 64],
                                bvb[:, hh * 64:(hh + 1) * 64], OP.add)

            def attention(l, qc):
                par = par_pool[0]
                cs = slice(qc * 512, (qc + 1) * 512)
                qt = [qap.tile([P, 512], FP16, tag=f"q{i}", name=f"q{i}")
                      for i in range(ET)]
                at = [aop.tile([P, 512], FP16, tag=f"a{i}", name=f"a{i}")
                      for i in range(ET)]
                rhs_x = [xt[k][:, cs] for k in range(ET)]

                def ev_q(m, ps):
                    nc.vector.tensor_scalar_add(qt[m][:], ps[:],
                                                par[:, m:m + 1])
                gemm(wqkvT[l], 0, ET, rhs_x, 2, ev_q, "wa")

                for hp_ in range(NH // 2):
                    es8 = []
                    for t in range(KT):
                        pss = psc.tile([P, 1024], F32, tag="sc", name="sc")
                        for hi, off in ((0, 0), (1, 64)):
                            nc.tensor.matmul(
                                pss[:, hi * 512:(hi + 1) * 512],
                                kt[hp_][off:off + 64, P * t:P * (t + 1)],
                                qt[hp_][off:off + 64, :],
                                start=True, stop=True)
                        es = esp.tile([P, 1024], FP16, tag="es", name="es")
                        nc.scalar.activation(es[:], pss[:], AF.Exp,
                                             scale=0.125)
                        es8.append(es)
                    for hi in range(2):
                        h = 2 * hp_ + hi
                        po = ppo.tile([P, 512], F32, tag="po", name="po")
                        for t in range(KT):
                            nc.tensor.matmul(
                                po[0:DH + 1, :],
                                vp[t][:, h * 65:(h + 1) * 65],
                                es8[t][:, hi * 512:(hi + 1) * 512],
                                start=(t == 0), stop=(t == KT - 1))
                        # two copies drain the bank; recip needs an SBUF
                        # input at base_partition 0 (else garbage)
                        poS = scp.tile([DH, 512], F32, tag="poS", name="poS")
                        nc.vector.tensor_copy(poS[:], po[0:DH, :])
                        den = scp.tile([1, 512], F32, tag="den", name="den")
                        nc.vector.tensor_copy(den[:], po[DH:DH + 1, :])
                        rc = scp.tile([1, 512], F32, tag="rc", name="rc")
                        nc.vector.reciprocal_approx_fast(
                            out=rc[:], in_=den[:])
                        bct = scp.tile([DH, 512], F32, tag="bct", name="bct")
                        nc.gpsimd.partition_broadcast(bct[:], rc[:])
                        nc.vector.tensor_tensor(
                            at[hp_][hi * 64:hi * 64 + 64, :],
                            poS[:], bct[:], OP.mult)

                def ev_o(m, ps):
                    tmp = scp.tile([P, 512], FP16, tag="tmp", name="tmp")
                    nc.vector.tensor_scalar_add(tmp[:], ps[:],
                                                par[:, 24 + m:25 + m])
                    nc.vector.tensor_tensor(xt[m][:, cs], xt[m][:, cs],
                                            tmp[:], OP.add)
                gemm(woT[l], 0, ET, [a[:] for a in at], 2, ev_o, "wa")

            def layer_norm(qc, g_col, b_col):
                """Post-norm LN on xt[:, qc-chunk] in place (fp16)."""
                par = par_pool[0]
                cs = slice(qc * 512, (qc + 1) * 512)
                s1 = pjp.tile([P, 512], F32, tag="pj", name="pj")
                for k in range(ET):
                    nc.tensor.matmul(s1[0:1, :], ones[:], xt[k][:, cs],
                                     start=(k == 0), stop=(k == ET - 1))
                s2 = pjp.tile([P, 512], F32, tag="pj", name="pj")
                for k in range(ET):
                    sq = lnp.tile([P, 512], FP16, tag="sq", name="sq")
                    nc.scalar.activation(sq[:], xt[k][:, cs], AF.Square)
                    nc.tensor.matmul(s2[0:1, :], ones[:], sq[:],
                                     start=(k == 0), stop=(k == ET - 1))
                m1 = lnp.tile([1, 512], FP16, tag="m1", name="m1", bufs=1)
                nc.vector.tensor_scalar_mul(m1[:], s1[0:1, :], 1.0 / E)
                m2 = lnp.tile([1, 512], F32, tag="m2", name="m2", bufs=1)
                nc.vector.tensor_scalar_mul(m2[:], s2[0:1, :], 1.0 / E)
                # var = m2 - m1^2
                msq = lnp.tile([1, 512], F32, tag="msq", name="msq", bufs=1)
                nc.vector.tensor_tensor(msq[:], m1[:], m1[:], OP.mult)
                nc.vector.tensor_tensor(m2[:], m2[:], msq[:], OP.subtract)
                lnv = lnp.tile([1, 512], F32, tag="lnv", name="lnv", bufs=1)
                nc.scalar.activation(lnv[:], m2[:], AF.Ln, bias=eps_t[0:1, :])
                mb = lnp.tile([P, 512], FP16, tag="mb", name="mb")
                nc.gpsimd.partition_broadcast(mb[:], m1[:])
                lnb = lnp.tile([P, 512], F32, tag="lnb", name="lnb")
                nc.gpsimd.partition_broadcast(lnb[:], lnv[:])
                vb = lnp.tile([P, 512], FP16, tag="vb", name="vb")
                nc.scalar.activation(vb[:], lnb[:], AF.Exp, scale=-0.5)
                for k in range(ET):
                    t1 = lnp.tile([P, 512], FP16, tag="t1", name="t1")
                    nc.vector.tensor_tensor(t1[:], xt[k][:, cs], mb[:],
                                            OP.subtract)
                    nc.vector.tensor_tensor(t1[:], t1[:], vb[:], OP.mult)
                    nc.vector.tensor_scalar(
                        xt[k][:, cs], t1[:],
                        par[:, g_col + k:g_col + k + 1],
                        par[:, b_col + k:b_col + k + 1],
                        OP.mult, OP.add)

            def ffn(l, qc):
                par = par_pool[0]
                cs = slice(qc * 512, (qc + 1) * 512)
                ht = [hp.tile([P, 512], FP16, tag=f"h{i}", name=f"h{i}")
                      for i in range(FT)]
                rhs_x = [xt[k][:, cs] for k in range(ET)]

                def ev_h(m, ps):
                    # DVE, not ACT: ACT is saturated by softmax exp and a
                    # relu eviction there stalls FFN1's psum rotation
                    nc.vector.tensor_scalar(
                        ht[m][:], ps[:], par[:, 32 + m:33 + m], 0.0,
                        OP.add, OP.max)
                gemm(w1T[l], 0, ET, rhs_x, 8, ev_h, "wf")

                def ev_f2(m, ps):
                    tmp = scp.tile([P, 512], FP16, tag="tmp", name="tmp")
                    nc.vector.tensor_scalar_add(tmp[:], ps[:],
                                                par[:, 64 + m:65 + m])
                    nc.vector.tensor_tensor(xt[m][:, cs], xt[m][:, cs],
                                            tmp[:], OP.add)
                gemm(w2T[l], 0, FT, [h[:] for h in ht], 2, ev_f2, "wf")

            # ---------------- main schedule ----------------
            with tc.tile_pool(name="parp", bufs=2) as parp:
                def load_par(l):
                    par = parp.tile([P, 104], F32, tag="par", name="par")
                    nc.sync.dma_start(par[:], params[l])
                    par_pool[0] = par

                load_par(0)
                kv_proj(0)
                simple = skip_attn or skip_ffn or skip_ln
                if simple:
                    for l in range(nlayers):
                        for qc in range(NQ):
                            if not skip_attn:
                                attention(l, qc)
                            par_next = None
                            if qc == NQ - 1 and l + 1 < nlayers:
                                par_l = par_pool[0]
                                load_par(l + 1)
                                par_next = par_pool[0]
                                kv_proj(l + 1)
                                par_pool[0] = par_l
                            if not skip_ln:
                                layer_norm(qc, 72, 80)
                            if not skip_ffn:
                                ffn(l, qc)
                                if not skip_ln:
                                    layer_norm(qc, 88, 96)
                            if par_next is not None:
                                par_pool[0] = par_next
                else:
                    # software pipeline: attention(next) issues ahead of
                    # ffn(cur) so FFN matmuls fill attention's exp/PV stalls
                    attention(0, 0)
                    for l in range(nlayers):
                        for qc in range(NQ):
                            par_next = None
                            if qc + 1 < NQ:
                                attention(l, qc + 1)
                            elif l + 1 < nlayers:
                                par_l = par_pool[0]
                                load_par(l + 1)
                                par_next = par_pool[0]
                                kv_proj(l + 1)
                                attention(l + 1, 0)
                                par_pool[0] = par_l
                            layer_norm(qc, 72, 80)
                            ffn(l, qc)
                            layer_norm(qc, 88, 96)
                            if par_next is not None:
                                par_pool[0] = par_next

            for i in range(ET):
                nc.sync.dma_start(outT[P * i:P * (i + 1), :], xt[i][:])

    nc.compile()
    return nc


def _prep_inputs(inputs):
    """Host-side: transpose weights to fp16 / pack params; per-core in_maps."""
    emb = np.asarray(inputs["embeddings"], dtype=np.float32)
    ipw = np.asarray(inputs["in_proj_w"], dtype=np.float32)   # [L, 3E, E]
    ipb = np.asarray(inputs["in_proj_b"], dtype=np.float32)   # [L, 3E]
    ow = np.asarray(inputs["out_w"], dtype=np.float32)        # [L, E, E]
    ob = np.asarray(inputs["out_b"], dtype=np.float32)        # [L, E]
    l1w = np.asarray(inputs["lin1_w"], dtype=np.float32)      # [L, DFF, E]
    l1b = np.asarray(inputs["lin1_b"], dtype=np.float32)      # [L, DFF]
    l2w = np.asarray(inputs["lin2_w"], dtype=np.float32)      # [L, E, DFF]
    l2b = np.asarray(inputs["lin2_b"], dtype=np.float32)      # [L, E]
    g1 = np.asarray(inputs["ln1_w"], dtype=np.float32)
    be1 = np.asarray(inputs["ln1_b"], dtype=np.float32)
    g2 = np.asarray(inputs["ln2_w"], dtype=np.float32)
    be2 = np.asarray(inputs["ln2_b"], dtype=np.float32)

    wqkvT = np.ascontiguousarray(ipw.transpose(0, 2, 1)).astype(np.float16)
    woT = np.ascontiguousarray(ow.transpose(0, 2, 1)).astype(np.float16)
    w1T = np.ascontiguousarray(l1w.transpose(0, 2, 1)).astype(np.float16)
    w2T = np.ascontiguousarray(l2w.transpose(0, 2, 1)).astype(np.float16)

    def cols(a, n):  # [L, n*128] -> [L, 128, n]
        return a.reshape(L, n, P).transpose(0, 2, 1)

    params = np.concatenate([
        cols(ipb[:, 0:E], 8), cols(ipb[:, E:2 * E], 8), cols(ipb[:, 2 * E:], 8),
        cols(ob, 8), cols(l1b, 32), cols(l2b, 8),
        cols(g1, 8), cols(be1, 8), cols(g2, 8), cols(be2, 8),
    ], axis=2)
    params = np.ascontiguousarray(params, dtype=np.float32)   # [L, 128, 104]
    bvrow = np.ascontiguousarray(ipb[:, 2 * E:3 * E])         # [L, E]

    shared = dict(wqkvT=wqkvT, woT=woT, w1T=w1T, w2T=w2T,
                  params=params, bvrow=bvrow,
                  onesh=np.ones((P, 16), np.float16))
    in_maps = []
    for c in range(B):
        m = dict(shared)
        m["xT"] = np.ascontiguousarray(emb[c].T).astype(np.float16)  # [E, S]
        in_maps.append(m)
    return in_maps


_NC_CACHE = {}


def _get_nc():
    if "nc" not in _NC_CACHE:
        _NC_CACHE["nc"] = build_encoder()
    return _NC_CACHE["nc"]


def run(inputs, trace=False, tmpdir=None):
    """Run on 8 NeuronCores; returns (output [8, S, E], BassKernelResults)."""
    in_maps = _prep_inputs(inputs)
    nc = _get_nc()
    res = bass_utils.run_bass_kernel_spmd(
        nc, in_maps, core_ids=list(range(B)), trace=trace, tmpdir=tmpdir)
    out = np.stack([np.ascontiguousarray(res.results[c]["outT"].T)
                    for c in range(B)]).astype(np.float32)
    return out, res


def kernel(**inputs):
    num_ctx = int(np.asarray(inputs["num_ctx"]))
    assert num_ctx == NC, f"kernel hardcodes num_ctx={NC}, got {num_ctx}"
    out, _ = run(inputs)
    return out
